# revision 1
# baseline (speedup 1.0000x reference)
"""CFN cell on 8 TRN2 NeuronCores — tensor-parallel over H, fp32r matmuls.

Same compute structure as v3 (acts stationary, [Wtu|Weu] / [Wtw|Wew]
fused 512-wide moving operands, 1536 fp32r MMs/core), but every large
DRAM tensor is pre-packed on the host so each SBUF partition's data is
contiguous in DRAM:

    acts:    [n_win, 128p, kt, 256]   -> 8-16 KB DMA descriptors
    weights: [128p, kt, h2]           -> 8-32 KB descriptors
    sbh:     [n_win, 128p, 2, h_loc]

This lifts per-queue DMA throughput ~4x (descriptor-rate limited at 1 KB
lines) which removes the startup stall and mid-kernel PE starvation.
"""

import numpy as np
from contextlib import ExitStack

import concourse.bass as bass
import concourse.mybir as mybir
import concourse.tile as tile
from concourse import bacc
from concourse.bass_utils import run_bass_kernel_spmd

F32 = mybir.dt.float32
F32R = mybir.dt.float32r
AF = mybir.ActivationFunctionType
ALU = mybir.AluOpType

B, D_IN, H, NCORES = 4096, 2048, 2048, 8
H_LOC = H // NCORES  # 256
WIN = 256

TRACE = False
LAST_RESULTS = None
_NC_CACHE = {}


def build(nc, b, d_in, d_state, h_loc, act_bufs=3, pipe=5):
    n_bt = b // 128
    ktx, kts = d_in // 128, d_state // 128
    h2 = 2 * h_loc
    tpw = WIN // 128
    n_win = b // WIN

    xt = nc.dram_tensor("xt", [n_win, 128, ktx, WIN], F32R,
                        kind="ExternalInput").ap()
    st = nc.dram_tensor("st", [n_win, 128, kts, WIN], F32R,
                        kind="ExternalInput").ap()
    sbh = nc.dram_tensor("sbh", [n_win, 128, tpw, h_loc], F32,
                         kind="ExternalInput").ap()
    wte = nc.dram_tensor("wte", [128, kts, h2], F32R, kind="ExternalInput").ap()
    wtwe = nc.dram_tensor("wtwe", [128, ktx, h2], F32R, kind="ExternalInput").ap()
    wwx = nc.dram_tensor("wwx", [128, ktx, h_loc], F32R, kind="ExternalInput").ap()
    bias = nc.dram_tensor("bias", [h2], F32, kind="ExternalInput").ap()
    out = nc.dram_tensor("h_out", [b, h_loc], F32, kind="ExternalOutput").ap()

    with tile.TileContext(nc) as tc, ExitStack() as ctx:
        consts = ctx.enter_context(tc.tile_pool(name="consts", bufs=1))
        acts = ctx.enter_context(tc.tile_pool(name="acts", bufs=act_bufs))
        temps = ctx.enter_context(tc.tile_pool(name="temps", bufs=2))
        psum = ctx.enter_context(tc.tile_pool(name="psum", bufs=1, space="PSUM"))

        stw_map, xtw_map, sbh_map = {}, {}, {}

        def load_stw(iw, nq=2, eng=None):
            eng = eng or nc.sync
            stw = acts.tile([128, kts, WIN], F32R, tag="stw", name=f"stw{iw}")
            step = max(kts // nq, 1)
            for c in range(0, kts, step):
                ce = min(c + step, kts)
                eng.dma_start(out=stw[:, c:ce, :], in_=st[iw, :, c:ce, :])
            stw_map[iw] = stw

        def load_xtw(iw, nq=2, eng=None):
            eng = eng or nc.sync
            xtw = acts.tile([128, ktx, WIN], F32R, tag="xtw", name=f"xtw{iw}")
            step = max(ktx // nq, 1)
            for c in range(0, ktx, step):
                ce = min(c + step, ktx)
                eng.dma_start(out=xtw[:, c:ce, :], in_=xt[iw, :, c:ce, :])
            xtw_map[iw] = xtw

        def load_sbh(iw):
            sbh_t = acts.tile([128, tpw, h_loc], F32, tag="sbh_t", name=f"sbh{iw}")
            nc.gpsimd.dma_start(out=sbh_t, in_=sbh[iw])
            sbh_map[iw] = sbh_t

        # Startup choreography: feed the PE in consumption order.  s-phases
        # need stw + wte only, so those stream first (spread wide across
        # queues); the x-phase operands follow while the prologue s-phases
        # keep the PE busy.
        wte_sb = consts.tile([128, kts, h2], F32R, tag="wte")
        wtwe_sb = consts.tile([128, ktx, h2], F32R, tag="wtwe")
        wwx_sb = consts.tile([128, ktx, h_loc], F32R, tag="wwx")

        # s-phase consumption order: stw0, then wte chunks interleaved with
        # the stw windows the prologue s-phases will need next
        load_stw(0, nq=4)
        n_pre_win = min(act_bufs, n_win)
        pre_stw = []
        for iw in range(1, n_pre_win):
            stw = acts.tile([128, kts, WIN], F32R, tag="stw", name=f"stw{iw}")
            stw_map[iw] = stw
            pre_stw.append((iw, stw))
        for c in range(0, kts, 2):
            ce = min(c + 2, kts)
            nc.sync.dma_start(out=wte_sb[:, c:ce, :], in_=wte[:, c:ce, :])
            for iw, stw in pre_stw:
                nc.sync.dma_start(out=stw[:, c:ce, :], in_=st[iw, :, c:ce, :])
        bias_bc = consts.tile([128, h2], F32, tag="bias_bc")
        bias_bcast_ap = bass.AP(
            tensor=bias.tensor, offset=bias.offset,
            ap=[[0, 128]] + list(bias.ap),
        )
        nc.gpsimd.dma_start(out=bias_bc, in_=bias_bcast_ap)
        load_sbh(0)
        load_xtw(0, nq=4)
        for c in range(0, ktx, 2):
            ce = min(c + 2, ktx)
            nc.sync.dma_start(out=wtwe_sb[:, c:ce, :], in_=wtwe[:, c:ce, :])
            nc.sync.dma_start(out=wwx_sb[:, c:ce, :], in_=wwx[:, c:ce, :])
        for iw in range(1, n_pre_win):
            load_xtw(iw)
            load_sbh(iw)

        def get_window(iw):
            if iw not in stw_map:
                load_stw(iw)
                load_xtw(iw)
                load_sbh(iw)
            for m in (stw_map, xtw_map, sbh_map):
                for key in [k for k in m if k < iw - act_bufs]:
                    del m[key]
            return stw_map[iw], xtw_map[iw], sbh_map[iw]

        def s_phase(ib):
            stw, _, _ = get_window(ib // tpw)
            bcol = (ib % tpw) * 128
            ps_te = psum.tile([128, h2], F32, tag="ps_te", bufs=pipe + 1,
                              name=f"ps_te{ib}")
            for k in range(kts):
                nc.tensor.matmul(
                    ps_te, stw[:, k, bcol:bcol + 128], wte_sb[:, k, :],
                    start=(k == 0), stop=False,
                )
            return ps_te

        def x_phase_and_epilogue(ib, ps_te):
            bsl = slice(ib * 128, (ib + 1) * 128)
            _, xtw, sbh_t = get_window(ib // tpw)
            it = ib % tpw
            bcol = it * 128
            ps_w = psum.tile([128, h_loc], F32, tag="ps_w", bufs=2,
                             name=f"ps_w{ib}")
            for k in range(ktx):
                nc.tensor.matmul(
                    ps_te, xtw[:, k, bcol:bcol + 128], wtwe_sb[:, k, :],
                    start=False, stop=(k == ktx - 1),
                )
                nc.tensor.matmul(
                    ps_w, xtw[:, k, bcol:bcol + 128], wwx_sb[:, k, :],
                    start=(k == 0), stop=(k == ktx - 1),
                )

            tsh = temps.tile([128, h_loc], F32, tag="tsh", name=f"tsh{ib}")
            nc.scalar.activation(tsh, sbh_t[:, it, :], AF.Tanh)
            pre = temps.tile([128, h2], F32, tag="pre", name=f"pre{ib}")
            nc.vector.scalar_tensor_tensor(
                pre, ps_te, 1.0, bias_bc, op0=ALU.mult, op1=ALU.add,
            )
            theta = temps.tile([128, h_loc], F32, tag="theta", name=f"th{ib}")
            nc.scalar.activation(theta, pre[:, :h_loc], AF.Sigmoid)
            eta = temps.tile([128, h_loc], F32, tag="eta", name=f"et{ib}")
            nc.scalar.activation(eta, pre[:, h_loc:], AF.Sigmoid)
            twx = temps.tile([128, h_loc], F32, tag="twx", name=f"twx{ib}")
            nc.scalar.activation(twx, ps_w, AF.Tanh)

            p1 = temps.tile([128, h_loc], F32, tag="p1", name=f"p1{ib}")
            nc.vector.tensor_mul(p1, theta, tsh)
            p2 = temps.tile([128, h_loc], F32, tag="p2", name=f"p2{ib}")
            nc.vector.tensor_mul(p2, eta, twx)
            ho = temps.tile([128, h_loc], F32, tag="ho", name=f"ho{ib}")
            nc.vector.tensor_add(ho, p1, p2)
            nc.gpsimd.dma_start(out=out[bsl, :], in_=ho)

        pending = [(ib, s_phase(ib)) for ib in range(min(pipe, n_bt))]
        for ib in range(pipe, n_bt):
            pib, ps = pending.pop(0)
            x_phase_and_epilogue(pib, ps)
            pending.append((ib, s_phase(ib)))
        for pib, ps in pending:
            x_phase_and_epilogue(pib, ps)

    nc.compile()
    return nc


def _get_nc():
    key = (B, D_IN, H, H_LOC)
    if key not in _NC_CACHE:
        nc = bacc.Bacc("TRN2", target_bir_lowering=False, debug=False,
                       num_devices=NCORES)
        _NC_CACHE[key] = build(nc, B, D_IN, H, H_LOC)
    return _NC_CACHE[key]


def _pack_acts(at):  # at: [D, B] (transposed activations)
    d, b_ = at.shape
    kt, n_win = d // 128, b_ // WIN
    # (t*128+p, iw*WIN+j) -> [iw, p, t, j]
    return np.ascontiguousarray(
        at.reshape(kt, 128, n_win, WIN).transpose(2, 1, 0, 3)
    )


def _pack_w(wm):  # wm: [D, h] -> [p, t, h]
    d, h = wm.shape
    kt = d // 128
    return np.ascontiguousarray(wm.reshape(kt, 128, h).transpose(1, 0, 2))


def make_in_maps(inputs):
    x = np.ascontiguousarray(np.asarray(inputs["inputs"], dtype=np.float32))
    s = np.ascontiguousarray(np.asarray(inputs["state"], dtype=np.float32))
    w = {
        k: np.asarray(inputs[k], dtype=np.float32)
        for k in ("theta_u_w", "theta_w_w", "eta_u_w", "eta_w_w", "wx_w")
    }
    bt_full = np.asarray(inputs["theta_w_b"], dtype=np.float32)
    be_full = np.asarray(inputs["eta_w_b"], dtype=np.float32)

    xt_p = _pack_acts(x.T)  # shared by all cores
    st_p = _pack_acts(s.T)
    n_win, tpw = B // WIN, WIN // 128

    in_maps = []
    for c in range(NCORES):
        hsl = slice(c * H_LOC, (c + 1) * H_LOC)
        sbh_c = np.ascontiguousarray(
            s[:, hsl].reshape(n_win, tpw, 128, H_LOC).transpose(0, 2, 1, 3)
        )
        in_maps.append({
            "xt": xt_p,
            "st": st_p,
            "sbh": sbh_c,
            "wte": _pack_w(np.concatenate(
                [w["theta_u_w"][:, hsl], w["eta_u_w"][:, hsl]], axis=1)),
            "wtwe": _pack_w(np.concatenate(
                [w["theta_w_w"][:, hsl], w["eta_w_w"][:, hsl]], axis=1)),
            "wwx": _pack_w(w["wx_w"][:, hsl]),
            "bias": np.ascontiguousarray(
                np.concatenate([bt_full[hsl], be_full[hsl]])
            ),
        })
    return in_maps


def kernel(**inputs):
    global LAST_RESULTS
    in_maps = make_in_maps(inputs)
    nc = _get_nc()
    res = run_bass_kernel_spmd(nc, in_maps, core_ids=list(range(NCORES)),
                               trace=TRACE)
    LAST_RESULTS = res

    h = np.empty((B, H), np.float32)
    for c in range(NCORES):
        h[:, c * H_LOC:(c + 1) * H_LOC] = res.results[c]["h_out"]
    return (h, h)



# revision 2
# speedup vs baseline: 1.0828x; 1.0828x over previous
"""CFN cell on 8 TRN2 NeuronCores — tensor-parallel over H, bf16 matmuls.

v4: weight-stationary orientation. Each core owns H_LOC=256 hidden
columns (2 tiles of 128). Stationary operand = weight chunk
[K=128, M=128 h-cols]; moving operand = transposed activations
[K=128, N=512 batch]. PSUM output is [h, batch], so

  * the theta/eta biases are per-partition -> folded into ACTIVATE,
  * tanh(state) comes from slices of the already-loaded st pack,
  * 1280 N=512 matmuls/core (vs 1536 mixed) — all at PE line rate.

All matmul operands are bf16 (host-converted): halves HBM traffic to
~39 MB/core and enables FWL weight loads; PSUM accumulation stays fp32
so the output error (~5e-4) is far inside the 2e-2 gate.
"""

import numpy as np
import ml_dtypes
from contextlib import ExitStack

import concourse.bass as bass
import concourse.mybir as mybir
import concourse.tile as tile
from concourse import bacc
from concourse.bass_utils import run_bass_kernel_spmd

F32 = mybir.dt.float32
BF16 = mybir.dt.bfloat16
AF = mybir.ActivationFunctionType

B, D_IN, H, NCORES = 4096, 2048, 2048, 8
H_LOC = H // NCORES          # 256 -> 2 output tiles of 128
BG = 512                     # batch-group width (PSUM bank limit)
N_BG = B // BG               # 8
KT = D_IN // 128             # 16 contraction chunks per operand side

TRACE = False
LAST_RESULTS = None
_NC_CACHE = {}


def build(nc):
    st = nc.dram_tensor("st", [N_BG, 128, KT, BG], BF16, kind="ExternalInput").ap()
    xt = nc.dram_tensor("xt", [N_BG, 128, KT, BG], BF16, kind="ExternalInput").ap()
    sts = nc.dram_tensor("sts", [N_BG, 128, 2, BG], BF16, kind="ExternalInput").ap()
    wsu = nc.dram_tensor("wsu", [128, KT, 512], BF16, kind="ExternalInput").ap()
    wsx = nc.dram_tensor("wsx", [128, KT, 512], BF16, kind="ExternalInput").ap()
    wx = nc.dram_tensor("wx", [128, KT, 256], BF16, kind="ExternalInput").ap()
    bias = nc.dram_tensor("bias", [128, 4], F32, kind="ExternalInput").ap()
    out = nc.dram_tensor("h_out", [N_BG, 128, 2, BG], F32, kind="ExternalOutput").ap()

    with tile.TileContext(nc) as tc, ExitStack() as ctx:
        consts = ctx.enter_context(tc.tile_pool(name="consts", bufs=1))
        acts = ctx.enter_context(tc.tile_pool(name="acts", bufs=3))
        temps = ctx.enter_context(tc.tile_pool(name="temps", bufs=2))
        psum = ctx.enter_context(tc.tile_pool(name="psum", bufs=1, space="PSUM"))

        wsu_sb = consts.tile([128, KT, 512], BF16, tag="wsu")
        wsx_sb = consts.tile([128, KT, 512], BF16, tag="wsx")
        wx_sb = consts.tile([128, KT, 256], BF16, tag="wx")
        bias_sb = consts.tile([128, 4], F32, tag="bias")

        st_map, xt_map, sts_map = {}, {}, {}

        def load_window(g, chunks_st=((0, 8), (8, 16)), chunks_xt=((0, 8), (8, 16))):
            stw = acts.tile([128, KT, BG], BF16, tag="stw", name=f"stw{g}")
            for c0, c1 in chunks_st:
                nc.sync.dma_start(out=stw[:, c0:c1, :], in_=st[g, :, c0:c1, :])
            st_map[g] = stw
            xtw = acts.tile([128, KT, BG], BF16, tag="xtw", name=f"xtw{g}")
            for c0, c1 in chunks_xt:
                nc.sync.dma_start(out=xtw[:, c0:c1, :], in_=xt[g, :, c0:c1, :])
            xt_map[g] = xtw
            stsw = acts.tile([128, 2, BG], BF16, tag="stsw", name=f"sts{g}")
            nc.gpsimd.dma_start(out=stsw, in_=sts[g])
            sts_map[g] = stsw

        # ── Startup choreography ────────────────────────────────────────
        # The first MMs consume (wsu[k], st0[k]) pairs in k order; issue
        # those in matching fine chunks on two queue engines so the PE can
        # start within ~2 chunks of DMA landing.  The x-side operands and
        # later windows stream behind them.
        fine = ((0, 2), (2, 4), (4, 8), (8, 12), (12, 16))
        stw0 = acts.tile([128, KT, BG], BF16, tag="stw", name="stw0")
        st_map[0] = stw0
        for c0, c1 in fine:
            nc.sync.dma_start(out=stw0[:, c0:c1, :], in_=st[0, :, c0:c1, :])
            nc.gpsimd.dma_start(out=wsu_sb[:, c0:c1, :], in_=wsu[:, c0:c1, :])
        xtw0 = acts.tile([128, KT, BG], BF16, tag="xtw", name="xtw0")
        xt_map[0] = xtw0
        coarse = ((0, 4), (4, 8), (8, 12), (12, 16))
        for c0, c1 in coarse:
            nc.sync.dma_start(out=xtw0[:, c0:c1, :], in_=xt[0, :, c0:c1, :])
            nc.gpsimd.dma_start(out=wsx_sb[:, c0:c1, :], in_=wsx[:, c0:c1, :])
        nc.gpsimd.dma_start(out=wx_sb[:, 0:8, :], in_=wx[:, 0:8, :])
        nc.gpsimd.dma_start(out=wx_sb[:, 8:16, :], in_=wx[:, 8:16, :])
        nc.gpsimd.dma_start(out=bias_sb, in_=bias)
        stsw0 = acts.tile([128, 2, BG], BF16, tag="stsw", name="sts0")
        nc.gpsimd.dma_start(out=stsw0, in_=sts[0])
        sts_map[0] = stsw0
        load_window(1)
        load_window(2)

        def group(g, ht):
            stw, xtw, stsw = st_map[g], xt_map[g], sts_map[g]
            hsl = slice(ht * 128, (ht + 1) * 128)
            esl = slice(256 + ht * 128, 256 + (ht + 1) * 128)
            th_ps = psum.tile([128, BG], F32, tag="th", bufs=2, name=f"th{g}_{ht}")
            et_ps = psum.tile([128, BG], F32, tag="et", bufs=2, name=f"et{g}_{ht}")
            for k in range(KT):
                nc.tensor.matmul(th_ps, wsu_sb[:, k, hsl], stw[:, k, :],
                                 start=(k == 0), stop=False)
            for k in range(KT):
                nc.tensor.matmul(et_ps, wsu_sb[:, k, esl], stw[:, k, :],
                                 start=(k == 0), stop=False)
            for k in range(KT):
                nc.tensor.matmul(th_ps, wsx_sb[:, k, hsl], xtw[:, k, :],
                                 start=False, stop=(k == KT - 1))
            for k in range(KT):
                nc.tensor.matmul(et_ps, wsx_sb[:, k, esl], xtw[:, k, :],
                                 start=False, stop=(k == KT - 1))
            wx_ps = psum.tile([128, BG], F32, tag="wx", bufs=2, name=f"wx{g}_{ht}")
            for k in range(KT):
                nc.tensor.matmul(wx_ps, wx_sb[:, k, hsl], xtw[:, k, :],
                                 start=(k == 0), stop=(k == KT - 1))

            ths = temps.tile([128, BG], F32, tag="ths", name=f"ths{g}_{ht}")
            nc.scalar.activation(ths, stsw[:, ht, :], AF.Tanh)
            th = temps.tile([128, BG], F32, tag="th_s", name=f"ths_{g}_{ht}")
            nc.scalar.activation(th, th_ps, AF.Sigmoid,
                                 bias=bias_sb[:, ht:ht + 1])
            p1 = temps.tile([128, BG], F32, tag="p1", name=f"p1{g}_{ht}")
            nc.vector.tensor_mul(p1, th, ths)
            et = temps.tile([128, BG], F32, tag="et_s", name=f"ets_{g}_{ht}")
            nc.scalar.activation(et, et_ps, AF.Sigmoid,
                                 bias=bias_sb[:, 2 + ht:3 + ht])
            twx = temps.tile([128, BG], F32, tag="twx", name=f"twx{g}_{ht}")
            nc.scalar.activation(twx, wx_ps, AF.Tanh)
            p2 = temps.tile([128, BG], F32, tag="p2", name=f"p2{g}_{ht}")
            nc.vector.tensor_mul(p2, et, twx)
            ho = temps.tile([128, BG], F32, tag="ho", name=f"ho{g}_{ht}")
            nc.vector.tensor_add(ho, p1, p2)
            nc.gpsimd.dma_start(out=out[g, :, ht, :], in_=ho)

        for g in range(N_BG):
            if g + 3 <= N_BG - 1:
                load_window(g + 3)
            for m in (st_map, xt_map, sts_map):
                for key in [k for k in m if k < g]:
                    del m[key]
            group(g, 0)
            group(g, 1)

    nc.compile()
    return nc


def _get_nc():
    key = (B, D_IN, H)
    if key not in _NC_CACHE:
        nc = bacc.Bacc("TRN2", target_bir_lowering=False, debug=False,
                       num_devices=NCORES)
        _NC_CACHE[key] = build(nc)
    return _NC_CACHE[key]


def _pack_acts(at):  # at: [D, B] transposed activations -> [n_bg, 128, KT, BG]
    d, b_ = at.shape
    return np.ascontiguousarray(
        at.reshape(KT, 128, N_BG, BG).transpose(2, 1, 0, 3)
    )


def _pack_w(wm):  # [D, h] -> [128, KT, h]
    d, h = wm.shape
    return np.ascontiguousarray(wm.reshape(KT, 128, h).transpose(1, 0, 2))


def make_in_maps(inputs):
    bf = ml_dtypes.bfloat16
    x = np.asarray(inputs["inputs"], dtype=np.float32)
    s = np.asarray(inputs["state"], dtype=np.float32)
    w = {k: np.asarray(inputs[k], dtype=np.float32)
         for k in ("theta_u_w", "theta_w_w", "eta_u_w", "eta_w_w", "wx_w")}
    bt_full = np.asarray(inputs["theta_w_b"], dtype=np.float32)
    be_full = np.asarray(inputs["eta_w_b"], dtype=np.float32)

    xt_p = _pack_acts(x.T.astype(bf))     # shared by all cores
    st_p = _pack_acts(s.T.astype(bf))

    in_maps = []
    for c in range(NCORES):
        hsl = slice(c * H_LOC, (c + 1) * H_LOC)
        # sts: this core's own hidden-state slice, [n_bg, 128, 2, BG]
        # element (g, p, ht, j) = state[g*BG+j, hsl.start + ht*128 + p]
        sts_c = np.ascontiguousarray(
            s[:, hsl].reshape(N_BG, BG, 2, 128).transpose(0, 3, 2, 1)
        ).astype(bf)
        bias_c = np.stack([
            bt_full[hsl][:128], bt_full[hsl][128:],
            be_full[hsl][:128], be_full[hsl][128:],
        ], axis=1).astype(np.float32)
        in_maps.append({
            "st": st_p,
            "xt": xt_p,
            "sts": sts_c,
            "wsu": _pack_w(np.concatenate(
                [w["theta_u_w"][:, hsl], w["eta_u_w"][:, hsl]], axis=1
            ).astype(bf)),
            "wsx": _pack_w(np.concatenate(
                [w["theta_w_w"][:, hsl], w["eta_w_w"][:, hsl]], axis=1
            ).astype(bf)),
            "wx": _pack_w(w["wx_w"][:, hsl].astype(bf)),
            "bias": np.ascontiguousarray(bias_c),
        })
    return in_maps


def kernel(**inputs):
    global LAST_RESULTS
    in_maps = make_in_maps(inputs)
    nc = _get_nc()
    res = run_bass_kernel_spmd(nc, in_maps, core_ids=list(range(NCORES)),
                               trace=TRACE)
    LAST_RESULTS = res

    h = np.empty((B, H), np.float32)
    for c in range(NCORES):
        o = res.results[c]["h_out"]  # [N_BG, 128, 2, BG]
        h[:, c * H_LOC:(c + 1) * H_LOC] = (
            o.transpose(0, 3, 2, 1).reshape(B, H_LOC)
        )
    return (h, h)


# revision 5
# speedup vs baseline: 1.0852x; 1.0022x over previous
"""CFN cell on 8 TRN2 NeuronCores — tensor-parallel over H, bf16 matmuls.

v4: weight-stationary orientation. Each core owns H_LOC=256 hidden
columns (2 tiles of 128). Stationary operand = weight chunk
[K=128, M=128 h-cols]; moving operand = transposed activations
[K=128, N=512 batch]. PSUM output is [h, batch], so

  * the theta/eta biases are per-partition -> folded into ACTIVATE,
  * tanh(state) comes from slices of the already-loaded st pack,
  * 1280 N=512 matmuls/core (vs 1536 mixed) — all at PE line rate.

All matmul operands are bf16 (host-converted): halves HBM traffic to
~39 MB/core and enables FWL weight loads; PSUM accumulation stays fp32
so the output error (~5e-4) is far inside the 2e-2 gate.
"""

import numpy as np
import ml_dtypes
from contextlib import ExitStack

import concourse.bass as bass
import concourse.mybir as mybir
import concourse.tile as tile
from concourse import bacc
from concourse.bass_utils import run_bass_kernel_spmd

F32 = mybir.dt.float32
BF16 = mybir.dt.bfloat16
AF = mybir.ActivationFunctionType

B, D_IN, H, NCORES = 4096, 2048, 2048, 8
H_LOC = H // NCORES          # 256 -> 2 output tiles of 128
BG = 512                     # batch-group width (PSUM bank limit)
N_BG = B // BG               # 8
KT = D_IN // 128             # 16 contraction chunks per operand side

TRACE = False
LAST_RESULTS = None
_NC_CACHE = {}


def build(nc):
    st = nc.dram_tensor("st", [N_BG, 128, KT, BG], BF16, kind="ExternalInput").ap()
    xt = nc.dram_tensor("xt", [N_BG, 128, KT, BG], BF16, kind="ExternalInput").ap()
    sts = nc.dram_tensor("sts", [N_BG, 128, 2, BG], BF16, kind="ExternalInput").ap()
    wsu = nc.dram_tensor("wsu", [128, KT, 512], BF16, kind="ExternalInput").ap()
    wsx = nc.dram_tensor("wsx", [128, KT, 512], BF16, kind="ExternalInput").ap()
    wx = nc.dram_tensor("wx", [128, KT, 256], BF16, kind="ExternalInput").ap()
    bias = nc.dram_tensor("bias", [128, 4], F32, kind="ExternalInput").ap()
    out = nc.dram_tensor("h_out", [N_BG, 128, 2, BG], F32, kind="ExternalOutput").ap()

    with tile.TileContext(nc) as tc, ExitStack() as ctx:
        consts = ctx.enter_context(tc.tile_pool(name="consts", bufs=1))
        acts = ctx.enter_context(tc.tile_pool(name="acts", bufs=3))
        temps = ctx.enter_context(tc.tile_pool(name="temps", bufs=2))
        psum = ctx.enter_context(tc.tile_pool(name="psum", bufs=1, space="PSUM"))

        wsu_sb = consts.tile([128, KT, 512], BF16, tag="wsu")
        wsx_sb = consts.tile([128, KT, 512], BF16, tag="wsx")
        wx_sb = consts.tile([128, KT, 256], BF16, tag="wx")
        bias_sb = consts.tile([128, 4], F32, tag="bias")

        st_map, xt_map, sts_map = {}, {}, {}

        def load_window(g):
            # split each window across both queue engines to balance them
            xtw = acts.tile([128, KT, BG], BF16, tag="xtw", name=f"xtw{g}")
            nc.sync.dma_start(out=xtw[:, 0:8, :], in_=xt[g, :, 0:8, :])
            nc.gpsimd.dma_start(out=xtw[:, 8:16, :], in_=xt[g, :, 8:16, :])
            xt_map[g] = xtw
            stw = acts.tile([128, KT, BG], BF16, tag="stw", name=f"stw{g}")
            nc.sync.dma_start(out=stw[:, 0:8, :], in_=st[g, :, 0:8, :])
            nc.gpsimd.dma_start(out=stw[:, 8:16, :], in_=st[g, :, 8:16, :])
            st_map[g] = stw
            stsw = acts.tile([128, 2, BG], BF16, tag="stsw", name=f"sts{g}")
            nc.gpsimd.dma_start(out=stsw, in_=sts[g])
            sts_map[g] = stsw

        # ── Startup choreography ────────────────────────────────────────
        # Groups run their input phase first (θx, ηx, wx: ~280 GB/s demand)
        # and state phase second (~370 GB/s), so the prologue streams the
        # x-side operands first, in matching fine (act, weight) chunks on
        # the two queue engines, in exact consumption order.  Windows 2+
        # are issued from the main loop so they don't compete here.
        fine = ((0, 1), (1, 2), (2, 4), (4, 6), (6, 8), (8, 12), (12, 16))
        xtw0 = acts.tile([128, KT, BG], BF16, tag="xtw", name="xtw0")
        xt_map[0] = xtw0
        for c0, c1 in fine:
            nc.sync.dma_start(out=xtw0[:, c0:c1, :], in_=xt[0, :, c0:c1, :])
            nc.gpsimd.dma_start(out=wsx_sb[:, c0:c1, :], in_=wsx[:, c0:c1, :])
        nc.gpsimd.dma_start(out=wx_sb[:, 0:8, :], in_=wx[:, 0:8, :])
        nc.gpsimd.dma_start(out=wx_sb[:, 8:16, :], in_=wx[:, 8:16, :])
        stw0 = acts.tile([128, KT, BG], BF16, tag="stw", name="stw0")
        st_map[0] = stw0
        coarse = ((0, 4), (4, 8), (8, 12), (12, 16))
        for c0, c1 in coarse:
            nc.sync.dma_start(out=stw0[:, c0:c1, :], in_=st[0, :, c0:c1, :])
            nc.gpsimd.dma_start(out=wsu_sb[:, c0:c1, :], in_=wsu[:, c0:c1, :])
        nc.gpsimd.dma_start(out=bias_sb, in_=bias)
        stsw0 = acts.tile([128, 2, BG], BF16, tag="stsw", name="sts0")
        nc.gpsimd.dma_start(out=stsw0, in_=sts[0])
        sts_map[0] = stsw0
        load_window(1)

        def group(g, ht):
            stw, xtw, stsw = st_map[g], xt_map[g], sts_map[g]
            hsl = slice(ht * 128, (ht + 1) * 128)
            esl = slice(256 + ht * 128, 256 + (ht + 1) * 128)
            th_ps = psum.tile([128, BG], F32, tag="th", bufs=2, name=f"th{g}_{ht}")
            et_ps = psum.tile([128, BG], F32, tag="et", bufs=2, name=f"et{g}_{ht}")
            wx_ps = psum.tile([128, BG], F32, tag="wx", bufs=2, name=f"wx{g}_{ht}")
            for k in range(KT):
                nc.tensor.matmul(th_ps, wsx_sb[:, k, hsl], xtw[:, k, :],
                                 start=(k == 0), stop=False)
            for k in range(KT):
                nc.tensor.matmul(et_ps, wsx_sb[:, k, esl], xtw[:, k, :],
                                 start=(k == 0), stop=False)
            for k in range(KT):
                nc.tensor.matmul(wx_ps, wx_sb[:, k, hsl], xtw[:, k, :],
                                 start=(k == 0), stop=(k == KT - 1))
            for k in range(KT):
                nc.tensor.matmul(th_ps, wsu_sb[:, k, hsl], stw[:, k, :],
                                 start=False, stop=(k == KT - 1))
            for k in range(KT):
                nc.tensor.matmul(et_ps, wsu_sb[:, k, esl], stw[:, k, :],
                                 start=False, stop=(k == KT - 1))

            ths = temps.tile([128, BG], F32, tag="ths", name=f"ths{g}_{ht}")
            nc.scalar.activation(ths, stsw[:, ht, :], AF.Tanh)
            twx = temps.tile([128, BG], F32, tag="twx", name=f"twx{g}_{ht}")
            nc.scalar.activation(twx, wx_ps, AF.Tanh)
            th = temps.tile([128, BG], F32, tag="th_s", name=f"ths_{g}_{ht}")
            nc.scalar.activation(th, th_ps, AF.Sigmoid,
                                 bias=bias_sb[:, ht:ht + 1])
            p1 = temps.tile([128, BG], F32, tag="p1", name=f"p1{g}_{ht}")
            nc.vector.tensor_mul(p1, th, ths)
            et = temps.tile([128, BG], F32, tag="et_s", name=f"ets_{g}_{ht}")
            nc.scalar.activation(et, et_ps, AF.Sigmoid,
                                 bias=bias_sb[:, 2 + ht:3 + ht])
            p2 = temps.tile([128, BG], F32, tag="p2", name=f"p2{g}_{ht}")
            nc.vector.tensor_mul(p2, et, twx)
            ho = temps.tile([128, BG], F32, tag="ho", name=f"ho{g}_{ht}")
            nc.vector.tensor_add(ho, p1, p2)
            nc.gpsimd.dma_start(out=out[g, :, ht, :], in_=ho)

        for g in range(N_BG):
            if g + 2 <= N_BG - 1:
                load_window(g + 2)
            for m in (st_map, xt_map, sts_map):
                for key in [k for k in m if k < g]:
                    del m[key]
            group(g, 0)
            group(g, 1)

    nc.compile()
    return nc


def _get_nc():
    key = (B, D_IN, H)
    if key not in _NC_CACHE:
        nc = bacc.Bacc("TRN2", target_bir_lowering=False, debug=False,
                       num_devices=NCORES)
        _NC_CACHE[key] = build(nc)
    return _NC_CACHE[key]


def _pack_acts(at):  # at: [D, B] transposed activations -> [n_bg, 128, KT, BG]
    d, b_ = at.shape
    return np.ascontiguousarray(
        at.reshape(KT, 128, N_BG, BG).transpose(2, 1, 0, 3)
    )


def _pack_w(wm):  # [D, h] -> [128, KT, h]
    d, h = wm.shape
    return np.ascontiguousarray(wm.reshape(KT, 128, h).transpose(1, 0, 2))


def make_in_maps(inputs):
    bf = ml_dtypes.bfloat16
    x = np.asarray(inputs["inputs"], dtype=np.float32)
    s = np.asarray(inputs["state"], dtype=np.float32)
    w = {k: np.asarray(inputs[k], dtype=np.float32)
         for k in ("theta_u_w", "theta_w_w", "eta_u_w", "eta_w_w", "wx_w")}
    bt_full = np.asarray(inputs["theta_w_b"], dtype=np.float32)
    be_full = np.asarray(inputs["eta_w_b"], dtype=np.float32)

    xt_p = _pack_acts(x.T.astype(bf))     # shared by all cores
    st_p = _pack_acts(s.T.astype(bf))

    in_maps = []
    for c in range(NCORES):
        hsl = slice(c * H_LOC, (c + 1) * H_LOC)
        # sts: this core's own hidden-state slice, [n_bg, 128, 2, BG]
        # element (g, p, ht, j) = state[g*BG+j, hsl.start + ht*128 + p]
        sts_c = np.ascontiguousarray(
            s[:, hsl].reshape(N_BG, BG, 2, 128).transpose(0, 3, 2, 1)
        ).astype(bf)
        bias_c = np.stack([
            bt_full[hsl][:128], bt_full[hsl][128:],
            be_full[hsl][:128], be_full[hsl][128:],
        ], axis=1).astype(np.float32)
        in_maps.append({
            "st": st_p,
            "xt": xt_p,
            "sts": sts_c,
            "wsu": _pack_w(np.concatenate(
                [w["theta_u_w"][:, hsl], w["eta_u_w"][:, hsl]], axis=1
            ).astype(bf)),
            "wsx": _pack_w(np.concatenate(
                [w["theta_w_w"][:, hsl], w["eta_w_w"][:, hsl]], axis=1
            ).astype(bf)),
            "wx": _pack_w(w["wx_w"][:, hsl].astype(bf)),
            "bias": np.ascontiguousarray(bias_c),
        })
    return in_maps


def kernel(**inputs):
    global LAST_RESULTS
    in_maps = make_in_maps(inputs)
    nc = _get_nc()
    res = run_bass_kernel_spmd(nc, in_maps, core_ids=list(range(NCORES)),
                               trace=TRACE)
    LAST_RESULTS = res

    h = np.empty((B, H), np.float32)
    for c in range(NCORES):
        o = res.results[c]["h_out"]  # [N_BG, 128, 2, BG]
        h[:, c * H_LOC:(c + 1) * H_LOC] = (
            o.transpose(0, 3, 2, 1).reshape(B, H_LOC)
        )
    return (h, h)


# revision 7
# speedup vs baseline: 1.0896x; 1.0041x over previous
"""CFN cell on 8 TRN2 NeuronCores — tensor-parallel over H, bf16 matmuls.

v4: weight-stationary orientation. Each core owns H_LOC=256 hidden
columns (2 tiles of 128). Stationary operand = weight chunk
[K=128, M=128 h-cols]; moving operand = transposed activations
[K=128, N=512 batch]. PSUM output is [h, batch], so

  * the theta/eta biases are per-partition -> folded into ACTIVATE,
  * tanh(state) comes from slices of the already-loaded st pack,
  * 1280 N=512 matmuls/core (vs 1536 mixed) — all at PE line rate.

All matmul operands are bf16 (host-converted): halves HBM traffic to
~39 MB/core and enables FWL weight loads; PSUM accumulation stays fp32
so the output error (~5e-4) is far inside the 2e-2 gate.
"""

import numpy as np
import ml_dtypes
from contextlib import ExitStack

import concourse.bass as bass
import concourse.mybir as mybir
import concourse.tile as tile
from concourse import bacc
from concourse.bass_utils import run_bass_kernel_spmd

F32 = mybir.dt.float32
BF16 = mybir.dt.bfloat16
AF = mybir.ActivationFunctionType

B, D_IN, H, NCORES = 4096, 2048, 2048, 8
H_LOC = H // NCORES          # 256 -> 2 output tiles of 128
BG = 512                     # batch-group width (PSUM bank limit)
N_BG = B // BG               # 8
KT = D_IN // 128             # 16 contraction chunks per operand side

TRACE = False
LAST_RESULTS = None
_NC_CACHE = {}


def build(nc):
    st = nc.dram_tensor("st", [N_BG, 128, KT, BG], BF16, kind="ExternalInput").ap()
    xt = nc.dram_tensor("xt", [N_BG, 128, KT, BG], BF16, kind="ExternalInput").ap()
    sts = nc.dram_tensor("sts", [N_BG, 128, 2, BG], BF16, kind="ExternalInput").ap()
    wsu = nc.dram_tensor("wsu", [128, KT, 512], BF16, kind="ExternalInput").ap()
    wsx = nc.dram_tensor("wsx", [128, KT, 512], BF16, kind="ExternalInput").ap()
    wx = nc.dram_tensor("wx", [128, KT, 256], BF16, kind="ExternalInput").ap()
    bias = nc.dram_tensor("bias", [128, 4], F32, kind="ExternalInput").ap()
    out = nc.dram_tensor("h_out", [N_BG, 128, 2, BG], F32, kind="ExternalOutput").ap()

    with tile.TileContext(nc) as tc, ExitStack() as ctx:
        consts = ctx.enter_context(tc.tile_pool(name="consts", bufs=1))
        acts = ctx.enter_context(tc.tile_pool(name="acts", bufs=3))
        temps = ctx.enter_context(tc.tile_pool(name="temps", bufs=2))
        psum = ctx.enter_context(tc.tile_pool(name="psum", bufs=1, space="PSUM"))

        wsu_sb = consts.tile([128, KT, 512], BF16, tag="wsu")
        wsx_sb = consts.tile([128, KT, 512], BF16, tag="wsx")
        wx_sb = consts.tile([128, KT, 256], BF16, tag="wx")
        bias_sb = consts.tile([128, 4], F32, tag="bias")

        st_map, xt_map, sts_map = {}, {}, {}

        def load_window(g):
            # split each window across both queue engines to balance them
            xtw = acts.tile([128, KT, BG], BF16, tag="xtw", name=f"xtw{g}")
            nc.sync.dma_start(out=xtw[:, 0:8, :], in_=xt[g, :, 0:8, :])
            nc.gpsimd.dma_start(out=xtw[:, 8:16, :], in_=xt[g, :, 8:16, :])
            xt_map[g] = xtw
            stw = acts.tile([128, KT, BG], BF16, tag="stw", name=f"stw{g}")
            nc.sync.dma_start(out=stw[:, 0:8, :], in_=st[g, :, 0:8, :])
            nc.gpsimd.dma_start(out=stw[:, 8:16, :], in_=st[g, :, 8:16, :])
            st_map[g] = stw
            stsw = acts.tile([128, 2, BG], BF16, tag="stsw", name=f"sts{g}")
            nc.gpsimd.dma_start(out=stsw, in_=sts[g])
            sts_map[g] = stsw

        # ── Startup choreography ────────────────────────────────────────
        # Groups run their input phase first (θx, ηx, wx: ~280 GB/s demand)
        # and state phase second (~370 GB/s), so the prologue streams the
        # x-side operands first, in matching fine (act, weight) chunks on
        # the two queue engines, in exact consumption order.  Windows 2+
        # are issued from the main loop so they don't compete here.
        fine = ((0, 1), (1, 2), (2, 4), (4, 6), (6, 8), (8, 12), (12, 16))
        xtw0 = acts.tile([128, KT, BG], BF16, tag="xtw", name="xtw0")
        xt_map[0] = xtw0
        for c0, c1 in fine:
            nc.sync.dma_start(out=xtw0[:, c0:c1, :], in_=xt[0, :, c0:c1, :])
            nc.gpsimd.dma_start(out=wsx_sb[:, c0:c1, :], in_=wsx[:, c0:c1, :])
        nc.gpsimd.dma_start(out=wx_sb[:, 0:8, :], in_=wx[:, 0:8, :])
        nc.gpsimd.dma_start(out=wx_sb[:, 8:16, :], in_=wx[:, 8:16, :])
        stw0 = acts.tile([128, KT, BG], BF16, tag="stw", name="stw0")
        st_map[0] = stw0
        coarse = ((0, 4), (4, 8), (8, 12), (12, 16))
        for c0, c1 in coarse:
            nc.sync.dma_start(out=stw0[:, c0:c1, :], in_=st[0, :, c0:c1, :])
            nc.gpsimd.dma_start(out=wsu_sb[:, c0:c1, :], in_=wsu[:, c0:c1, :])
        nc.gpsimd.dma_start(out=bias_sb, in_=bias)
        stsw0 = acts.tile([128, 2, BG], BF16, tag="stsw", name="sts0")
        nc.gpsimd.dma_start(out=stsw0, in_=sts[0])
        sts_map[0] = stsw0
        load_window(1)

        def group(g):
            # Both h-tiles share every window / weight chunk, so running
            # them in one pass halves the DMA demand per PE-second — the
            # only thing that matters while the weights still stream in
            # (group 0 would otherwise need ~520 GB/s vs ~320 available).
            # ht-major sub-loops release each PSUM bank ~10 µs before the
            # group ends, so bufs=1 per tag never stalls a boundary.
            stw, xtw, stsw = st_map[g], xt_map[g], sts_map[g]
            hs = [slice(0, 128), slice(128, 256)]
            es = [slice(256, 384), slice(384, 512)]
            th_ps = [psum.tile([128, BG], F32, tag=f"th{ht}", bufs=1,
                               name=f"th{g}_{ht}") for ht in range(2)]
            et_ps = [psum.tile([128, BG], F32, tag=f"et{ht}", bufs=1,
                               name=f"et{g}_{ht}") for ht in range(2)]
            wx_ps = [psum.tile([128, BG], F32, tag=f"wx{ht}", bufs=1,
                               name=f"wx{g}_{ht}") for ht in range(2)]
            for ht in range(2):
                for k in range(KT):
                    nc.tensor.matmul(th_ps[ht], wsx_sb[:, k, hs[ht]],
                                     xtw[:, k, :], start=(k == 0), stop=False)
            for ht in range(2):
                for k in range(KT):
                    nc.tensor.matmul(et_ps[ht], wsx_sb[:, k, es[ht]],
                                     xtw[:, k, :], start=(k == 0), stop=False)
            for ht in range(2):
                for k in range(KT):
                    nc.tensor.matmul(wx_ps[ht], wx_sb[:, k, hs[ht]],
                                     xtw[:, k, :], start=(k == 0),
                                     stop=(k == KT - 1))
            for ht in range(2):
                for k in range(KT):
                    nc.tensor.matmul(th_ps[ht], wsu_sb[:, k, hs[ht]],
                                     stw[:, k, :], start=False,
                                     stop=(k == KT - 1))
            for ht in range(2):
                for k in range(KT):
                    nc.tensor.matmul(et_ps[ht], wsu_sb[:, k, es[ht]],
                                     stw[:, k, :], start=False,
                                     stop=(k == KT - 1))

            # epilogue, scalar-queue ops emitted in availability order so a
            # late PSUM (et) never head-of-line-blocks an earlier one
            ths, twx, th, p1 = [], [], [], []
            for ht in range(2):
                ths.append(temps.tile([128, BG], F32, tag="ths",
                                      name=f"ths{g}_{ht}"))
                nc.scalar.activation(ths[ht], stsw[:, ht, :], AF.Tanh)
            for ht in range(2):
                twx.append(temps.tile([128, BG], F32, tag="twx",
                                      name=f"twx{g}_{ht}"))
                nc.scalar.activation(twx[ht], wx_ps[ht], AF.Tanh)
            for ht in range(2):
                th.append(temps.tile([128, BG], F32, tag="th_s",
                                     name=f"ths_{g}_{ht}"))
                nc.scalar.activation(th[ht], th_ps[ht], AF.Sigmoid,
                                     bias=bias_sb[:, ht:ht + 1])
                p1.append(temps.tile([128, BG], F32, tag="p1",
                                     name=f"p1{g}_{ht}"))
                nc.vector.tensor_mul(p1[ht], th[ht], ths[ht])
            for ht in range(2):
                et = temps.tile([128, BG], F32, tag="et_s", name=f"ets_{g}_{ht}")
                nc.scalar.activation(et, et_ps[ht], AF.Sigmoid,
                                     bias=bias_sb[:, 2 + ht:3 + ht])
                p2 = temps.tile([128, BG], F32, tag="p2", name=f"p2{g}_{ht}")
                nc.vector.tensor_mul(p2, et, twx[ht])
                ho = temps.tile([128, BG], F32, tag="ho", name=f"ho{g}_{ht}")
                nc.vector.tensor_add(ho, p1[ht], p2)
                nc.gpsimd.dma_start(out=out[g, :, ht, :], in_=ho)

        for g in range(N_BG):
            if g + 2 <= N_BG - 1:
                load_window(g + 2)
            for m in (st_map, xt_map, sts_map):
                for key in [k for k in m if k < g]:
                    del m[key]
            group(g)

    nc.compile()
    return nc


def _get_nc():
    key = (B, D_IN, H)
    if key not in _NC_CACHE:
        nc = bacc.Bacc("TRN2", target_bir_lowering=False, debug=False,
                       num_devices=NCORES)
        _NC_CACHE[key] = build(nc)
    return _NC_CACHE[key]


def _pack_acts(at):  # at: [D, B] transposed activations -> [n_bg, 128, KT, BG]
    d, b_ = at.shape
    return np.ascontiguousarray(
        at.reshape(KT, 128, N_BG, BG).transpose(2, 1, 0, 3)
    )


def _pack_w(wm):  # [D, h] -> [128, KT, h]
    d, h = wm.shape
    return np.ascontiguousarray(wm.reshape(KT, 128, h).transpose(1, 0, 2))


def make_in_maps(inputs):
    bf = ml_dtypes.bfloat16
    x = np.asarray(inputs["inputs"], dtype=np.float32)
    s = np.asarray(inputs["state"], dtype=np.float32)
    w = {k: np.asarray(inputs[k], dtype=np.float32)
         for k in ("theta_u_w", "theta_w_w", "eta_u_w", "eta_w_w", "wx_w")}
    bt_full = np.asarray(inputs["theta_w_b"], dtype=np.float32)
    be_full = np.asarray(inputs["eta_w_b"], dtype=np.float32)

    xt_p = _pack_acts(x.T.astype(bf))     # shared by all cores
    st_p = _pack_acts(s.T.astype(bf))

    in_maps = []
    for c in range(NCORES):
        hsl = slice(c * H_LOC, (c + 1) * H_LOC)
        # sts: this core's own hidden-state slice, [n_bg, 128, 2, BG]
        # element (g, p, ht, j) = state[g*BG+j, hsl.start + ht*128 + p]
        sts_c = np.ascontiguousarray(
            s[:, hsl].reshape(N_BG, BG, 2, 128).transpose(0, 3, 2, 1)
        ).astype(bf)
        bias_c = np.stack([
            bt_full[hsl][:128], bt_full[hsl][128:],
            be_full[hsl][:128], be_full[hsl][128:],
        ], axis=1).astype(np.float32)
        in_maps.append({
            "st": st_p,
            "xt": xt_p,
            "sts": sts_c,
            "wsu": _pack_w(np.concatenate(
                [w["theta_u_w"][:, hsl], w["eta_u_w"][:, hsl]], axis=1
            ).astype(bf)),
            "wsx": _pack_w(np.concatenate(
                [w["theta_w_w"][:, hsl], w["eta_w_w"][:, hsl]], axis=1
            ).astype(bf)),
            "wx": _pack_w(w["wx_w"][:, hsl].astype(bf)),
            "bias": np.ascontiguousarray(bias_c),
        })
    return in_maps


def kernel(**inputs):
    global LAST_RESULTS
    in_maps = make_in_maps(inputs)
    nc = _get_nc()
    res = run_bass_kernel_spmd(nc, in_maps, core_ids=list(range(NCORES)),
                               trace=TRACE)
    LAST_RESULTS = res

    h = np.empty((B, H), np.float32)
    for c in range(NCORES):
        o = res.results[c]["h_out"]  # [N_BG, 128, 2, BG]
        h[:, c * H_LOC:(c + 1) * H_LOC] = (
            o.transpose(0, 3, 2, 1).reshape(B, H_LOC)
        )
    return (h, h)


# revision 9
# speedup vs baseline: 1.1037x; 1.0129x over previous
"""CFN cell on 8 TRN2 NeuronCores — tensor-parallel over H, bf16 matmuls.

v4: weight-stationary orientation. Each core owns H_LOC=256 hidden
columns (2 tiles of 128). Stationary operand = weight chunk
[K=128, M=128 h-cols]; moving operand = transposed activations
[K=128, N=512 batch]. PSUM output is [h, batch], so

  * the theta/eta biases are per-partition -> folded into ACTIVATE,
  * tanh(state) comes from slices of the already-loaded st pack,
  * 1280 N=512 matmuls/core (vs 1536 mixed) — all at PE line rate.

All matmul operands are bf16 (host-converted): halves HBM traffic to
~39 MB/core and enables FWL weight loads; PSUM accumulation stays fp32
so the output error (~5e-4) is far inside the 2e-2 gate.
"""

import numpy as np
import ml_dtypes
from contextlib import ExitStack

import concourse.bass as bass
import concourse.mybir as mybir
import concourse.tile as tile
from concourse import bacc
from concourse.bass_utils import run_bass_kernel_spmd

F32 = mybir.dt.float32
BF16 = mybir.dt.bfloat16
AF = mybir.ActivationFunctionType

B, D_IN, H, NCORES = 4096, 2048, 2048, 8
H_LOC = H // NCORES          # 256 -> 2 output tiles of 128
BG = 512                     # batch-group width (PSUM bank limit)
N_BG = B // BG               # 8
KT = D_IN // 128             # 16 contraction chunks per operand side

TRACE = False
LAST_RESULTS = None
_NC_CACHE = {}


def build(nc):
    st = nc.dram_tensor("st", [N_BG, 128, KT, BG], BF16, kind="ExternalInput").ap()
    xt = nc.dram_tensor("xt", [N_BG, 128, KT, BG], BF16, kind="ExternalInput").ap()
    sts = nc.dram_tensor("sts", [N_BG, 128, 2, BG], BF16, kind="ExternalInput").ap()
    wsu = nc.dram_tensor("wsu", [128, KT, 512], BF16, kind="ExternalInput").ap()
    wsx = nc.dram_tensor("wsx", [128, KT, 512], BF16, kind="ExternalInput").ap()
    wx = nc.dram_tensor("wx", [128, KT, 256], BF16, kind="ExternalInput").ap()
    bias = nc.dram_tensor("bias", [128, 4], F32, kind="ExternalInput").ap()
    out = nc.dram_tensor("h_out", [N_BG, 128, 2, BG], F32, kind="ExternalOutput").ap()

    with tile.TileContext(nc) as tc, ExitStack() as ctx:
        consts = ctx.enter_context(tc.tile_pool(name="consts", bufs=1))
        acts = ctx.enter_context(tc.tile_pool(name="acts", bufs=3))
        temps = ctx.enter_context(tc.tile_pool(name="temps", bufs=2))
        psum = ctx.enter_context(tc.tile_pool(name="psum", bufs=1, space="PSUM"))

        wsu_sb = consts.tile([128, KT, 512], BF16, tag="wsu")
        wsx_sb = consts.tile([128, KT, 512], BF16, tag="wsx")
        wx_sb = consts.tile([128, KT, 256], BF16, tag="wx")
        bias_sb = consts.tile([128, 4], F32, tag="bias")

        st_map, xt_map, sts_map = {}, {}, {}

        # DMA issue rings.  The 16 DMA engines pull from every active ring,
        # so aggregate HBM bandwidth scales with the number of rings kept
        # busy; round-robin in exact consumption order keeps delivery
        # aligned with what the PE needs next.  gpsimd is reserved for the
        # output writes in steady state (an out DMA waits on the epilogue
        # and would head-of-line-block window loads queued behind it).
        import itertools
        _rr = itertools.count()
        rings4 = [nc.sync, nc.gpsimd, nc.scalar]
        rings3 = [nc.sync, nc.scalar]

        def q4():
            return rings4[next(_rr) % 3]

        def load_window(g):
            xtw = acts.tile([128, KT, BG], BF16, tag="xtw", name=f"xtw{g}")
            for i, (c0, c1) in enumerate(((0, 8), (8, 16))):
                rings3[i].dma_start(out=xtw[:, c0:c1, :],
                                    in_=xt[g, :, c0:c1, :])
            xt_map[g] = xtw
            stw = acts.tile([128, KT, BG], BF16, tag="stw", name=f"stw{g}")
            for i, (c0, c1) in enumerate(((0, 8), (8, 16))):
                rings3[1 - i].dma_start(out=stw[:, c0:c1, :],
                                        in_=st[g, :, c0:c1, :])
            st_map[g] = stw
            stsw = acts.tile([128, 2, BG], BF16, tag="stsw", name=f"sts{g}")
            rings3[g % 2].dma_start(out=stsw, in_=sts[g])
            sts_map[g] = stsw

        # ── Startup choreography ────────────────────────────────────────
        # Consumption order: [xt0|wsx] pairs (input phase), wx, [st0|wsu]
        # pairs (state phase), then window 1.  Fine chunks at the front so
        # the first matmul can start ~1 chunk after DMA begins.
        fine = ((0, 1), (1, 2), (2, 4), (4, 6), (6, 8), (8, 12), (12, 16))
        xtw0 = acts.tile([128, KT, BG], BF16, tag="xtw", name="xtw0")
        xt_map[0] = xtw0
        for c0, c1 in fine:
            q4().dma_start(out=xtw0[:, c0:c1, :], in_=xt[0, :, c0:c1, :])
            q4().dma_start(out=wsx_sb[:, c0:c1, :], in_=wsx[:, c0:c1, :])
        q4().dma_start(out=wx_sb[:, 0:8, :], in_=wx[:, 0:8, :])
        q4().dma_start(out=wx_sb[:, 8:16, :], in_=wx[:, 8:16, :])
        stw0 = acts.tile([128, KT, BG], BF16, tag="stw", name="stw0")
        st_map[0] = stw0
        coarse = ((0, 2), (2, 4), (4, 6), (6, 8), (8, 12), (12, 16))
        for c0, c1 in coarse:
            q4().dma_start(out=stw0[:, c0:c1, :], in_=st[0, :, c0:c1, :])
            q4().dma_start(out=wsu_sb[:, c0:c1, :], in_=wsu[:, c0:c1, :])
        q4().dma_start(out=bias_sb, in_=bias)
        stsw0 = acts.tile([128, 2, BG], BF16, tag="stsw", name="sts0")
        q4().dma_start(out=stsw0, in_=sts[0])
        sts_map[0] = stsw0
        xtw1 = acts.tile([128, KT, BG], BF16, tag="xtw", name="xtw1")
        for c0, c1 in ((0, 8), (8, 16)):
            q4().dma_start(out=xtw1[:, c0:c1, :], in_=xt[1, :, c0:c1, :])
        xt_map[1] = xtw1
        stw1 = acts.tile([128, KT, BG], BF16, tag="stw", name="stw1")
        for c0, c1 in ((0, 8), (8, 16)):
            q4().dma_start(out=stw1[:, c0:c1, :], in_=st[1, :, c0:c1, :])
        st_map[1] = stw1
        stsw1 = acts.tile([128, 2, BG], BF16, tag="stsw", name="sts1")
        q4().dma_start(out=stsw1, in_=sts[1])
        sts_map[1] = stsw1

        def group(g):
            # Both h-tiles share every window / weight chunk, so running
            # them in one pass halves the DMA demand per PE-second — the
            # only thing that matters while the weights still stream in
            # (group 0 would otherwise need ~520 GB/s vs ~320 available).
            # ht-major sub-loops release each PSUM bank ~10 µs before the
            # group ends, so bufs=1 per tag never stalls a boundary.
            stw, xtw, stsw = st_map[g], xt_map[g], sts_map[g]
            hs = [slice(0, 128), slice(128, 256)]
            es = [slice(256, 384), slice(384, 512)]
            th_ps = [psum.tile([128, BG], F32, tag=f"th{ht}", bufs=1,
                               name=f"th{g}_{ht}") for ht in range(2)]
            et_ps = [psum.tile([128, BG], F32, tag=f"et{ht}", bufs=1,
                               name=f"et{g}_{ht}") for ht in range(2)]
            wx_ps = [psum.tile([128, BG], F32, tag=f"wx{ht}", bufs=1,
                               name=f"wx{g}_{ht}") for ht in range(2)]
            for ht in range(2):
                for k in range(KT):
                    nc.tensor.matmul(th_ps[ht], wsx_sb[:, k, hs[ht]],
                                     xtw[:, k, :], start=(k == 0), stop=False)
            for ht in range(2):
                for k in range(KT):
                    nc.tensor.matmul(et_ps[ht], wsx_sb[:, k, es[ht]],
                                     xtw[:, k, :], start=(k == 0), stop=False)
            for ht in range(2):
                for k in range(KT):
                    nc.tensor.matmul(wx_ps[ht], wx_sb[:, k, hs[ht]],
                                     xtw[:, k, :], start=(k == 0),
                                     stop=(k == KT - 1))
            for ht in range(2):
                for k in range(KT):
                    nc.tensor.matmul(th_ps[ht], wsu_sb[:, k, hs[ht]],
                                     stw[:, k, :], start=False,
                                     stop=(k == KT - 1))
            for ht in range(2):
                for k in range(KT):
                    nc.tensor.matmul(et_ps[ht], wsu_sb[:, k, es[ht]],
                                     stw[:, k, :], start=False,
                                     stop=(k == KT - 1))

            # epilogue, scalar-queue ops emitted in availability order so a
            # late PSUM (et) never head-of-line-blocks an earlier one
            ths, twx, th, p1 = [], [], [], []
            for ht in range(2):
                ths.append(temps.tile([128, BG], F32, tag="ths",
                                      name=f"ths{g}_{ht}"))
                nc.scalar.activation(ths[ht], stsw[:, ht, :], AF.Tanh)
            for ht in range(2):
                twx.append(temps.tile([128, BG], F32, tag="twx",
                                      name=f"twx{g}_{ht}"))
                nc.scalar.activation(twx[ht], wx_ps[ht], AF.Tanh)
            for ht in range(2):
                th.append(temps.tile([128, BG], F32, tag="th_s",
                                     name=f"ths_{g}_{ht}"))
                nc.scalar.activation(th[ht], th_ps[ht], AF.Sigmoid,
                                     bias=bias_sb[:, ht:ht + 1])
                p1.append(temps.tile([128, BG], F32, tag="p1",
                                     name=f"p1{g}_{ht}"))
                nc.vector.tensor_mul(p1[ht], th[ht], ths[ht])
            for ht in range(2):
                et = temps.tile([128, BG], F32, tag="et_s", name=f"ets_{g}_{ht}")
                nc.scalar.activation(et, et_ps[ht], AF.Sigmoid,
                                     bias=bias_sb[:, 2 + ht:3 + ht])
                p2 = temps.tile([128, BG], F32, tag="p2", name=f"p2{g}_{ht}")
                nc.vector.tensor_mul(p2, et, twx[ht])
                ho = temps.tile([128, BG], F32, tag="ho", name=f"ho{g}_{ht}")
                nc.vector.tensor_add(ho, p1[ht], p2)
                nc.gpsimd.dma_start(out=out[g, :, ht, :], in_=ho)

        for g in range(N_BG):
            if g + 2 <= N_BG - 1:
                load_window(g + 2)
            for m in (st_map, xt_map, sts_map):
                for key in [k for k in m if k < g]:
                    del m[key]
            group(g)

    nc.compile()
    return nc


def _get_nc():
    key = (B, D_IN, H)
    if key not in _NC_CACHE:
        nc = bacc.Bacc("TRN2", target_bir_lowering=False, debug=False,
                       num_devices=NCORES)
        _NC_CACHE[key] = build(nc)
    return _NC_CACHE[key]


def _pack_acts(at):  # at: [D, B] transposed activations -> [n_bg, 128, KT, BG]
    d, b_ = at.shape
    return np.ascontiguousarray(
        at.reshape(KT, 128, N_BG, BG).transpose(2, 1, 0, 3)
    )


def _pack_w(wm):  # [D, h] -> [128, KT, h]
    d, h = wm.shape
    return np.ascontiguousarray(wm.reshape(KT, 128, h).transpose(1, 0, 2))


def make_in_maps(inputs):
    bf = ml_dtypes.bfloat16
    x = np.asarray(inputs["inputs"], dtype=np.float32)
    s = np.asarray(inputs["state"], dtype=np.float32)
    w = {k: np.asarray(inputs[k], dtype=np.float32)
         for k in ("theta_u_w", "theta_w_w", "eta_u_w", "eta_w_w", "wx_w")}
    bt_full = np.asarray(inputs["theta_w_b"], dtype=np.float32)
    be_full = np.asarray(inputs["eta_w_b"], dtype=np.float32)

    xt_p = _pack_acts(x.T.astype(bf))     # shared by all cores
    st_p = _pack_acts(s.T.astype(bf))

    in_maps = []
    for c in range(NCORES):
        hsl = slice(c * H_LOC, (c + 1) * H_LOC)
        # sts: this core's own hidden-state slice, [n_bg, 128, 2, BG]
        # element (g, p, ht, j) = state[g*BG+j, hsl.start + ht*128 + p]
        sts_c = np.ascontiguousarray(
            s[:, hsl].reshape(N_BG, BG, 2, 128).transpose(0, 3, 2, 1)
        ).astype(bf)
        bias_c = np.stack([
            bt_full[hsl][:128], bt_full[hsl][128:],
            be_full[hsl][:128], be_full[hsl][128:],
        ], axis=1).astype(np.float32)
        in_maps.append({
            "st": st_p,
            "xt": xt_p,
            "sts": sts_c,
            "wsu": _pack_w(np.concatenate(
                [w["theta_u_w"][:, hsl], w["eta_u_w"][:, hsl]], axis=1
            ).astype(bf)),
            "wsx": _pack_w(np.concatenate(
                [w["theta_w_w"][:, hsl], w["eta_w_w"][:, hsl]], axis=1
            ).astype(bf)),
            "wx": _pack_w(w["wx_w"][:, hsl].astype(bf)),
            "bias": np.ascontiguousarray(bias_c),
        })
    return in_maps


def kernel(**inputs):
    global LAST_RESULTS
    in_maps = make_in_maps(inputs)
    nc = _get_nc()
    res = run_bass_kernel_spmd(nc, in_maps, core_ids=list(range(NCORES)),
                               trace=TRACE)
    LAST_RESULTS = res

    h = np.empty((B, H), np.float32)
    for c in range(NCORES):
        o = res.results[c]["h_out"]  # [N_BG, 128, 2, BG]
        h[:, c * H_LOC:(c + 1) * H_LOC] = (
            o.transpose(0, 3, 2, 1).reshape(B, H_LOC)
        )
    return (h, h)
